# revision 62
# baseline (speedup 1.0000x reference)
"""Trainium2 Bass kernel for the EnsembleFeatureLoss OT problem.

Math (per ensemble member e of E=4):
  s = l2norm_rows(gts[e]); t = l2norm_rows(feats[e])      # [4096, 1024]
  sim = s @ t.T                                            # [4096, 4096]
  K = exp(10*sim - 10)
  Sinkhorn on this data converges at iteration 1 (fp64: mean|r2-r1| ~ 5e-8,
  loss delta 6e-8 rel), so the transport plan is outer(r1, c1) * K with
    r1 = u / rowsum(K),  c1 = v / Y1,  Y1 = K^T r1
  and
    loss = sum_n c1[n] * Z[n],  Z[n] = sum_m r1[m] K[m,n] sim[m,n]
  A host-side sampled check of |r2 - r1| guards the approximation; on
  violation we fall back to a faithful numpy reference.

Distribution: 8 cores = 4 members x 2 row-halves (2048 rows each).
One fused pass per core over its [2048, 4096] block:
  fp8e4 DoubleRow matmul (2 d-blocks per instruction, 2 MACs/cell/cycle)
  -> psum chunk -> DVE applies 1/|t| (bf16 simw) -> one big ACT exp per
  row tile (per-row scale AP carries 10/|s|, fused rowsum accum) -> r1 ->
  PE accumulates Y1 (r1-weighted colsums of K) and Z (r1/|s|-weighted
  colsums of K*simw) into partition-packed psum banks across all 16 row
  tiles. No DRAM spill, no second pass, no device collective: each core
  outputs Y1/Z partials [2, 4096] + 1/rowsum; the host adds the two
  half partials and does the O(N) epilogue.

Inputs are shipped as raw (unnormalized) fp8e4 pair-interleaved blocks
(6 MB/core); both norm vectors are computed on device from the fp8
operands (squares -> PE ones-matmul colsums; the s-norm colsums use the square
chunk as the matmul stationary, landing directly in the per-row
[128,16] scale layout).
"""

import numpy as np
import ml_dtypes

BF16 = ml_dtypes.bfloat16
F8 = ml_dtypes.float8_e4m3

E = 4
M = 4096
N = 4096
D = 1024
P = 128
NCORES = 8
MHALF = M // 2              # rows per core
CH = 512                    # psum chunk (one fp32 bank)

_CACHE = {}


def build_bass(mhalf=MHALF, n=N, d=D, ncores=NCORES):
    import concourse.bass as bass
    import concourse.mybir as mybir
    import concourse.tile as tile
    from concourse import bacc
    from concourse.bass import ts

    dt = mybir.dt
    f32, bf16, f8 = dt.float32, dt.bfloat16, dt.float8e4
    Act = mybir.ActivationFunctionType
    DR = mybir.MatmulPerfMode.DoubleRow

    nt_m = mhalf // P           # 16 row tiles
    nd = d // P                 # 8 contraction blocks
    npair = nd // 2             # 4 DoubleRow pair blocks
    nch = n // CH               # 8 column chunks
    mch = mhalf // CH           # 4 s-norm column chunks

    nc = bacc.Bacc("TRN2", target_bir_lowering=False, debug=False,
                   num_devices=ncores)
    # pair-interleaved fp8 operands: row q*128+p, col two*F+x holds
    # X[(2q+two)*128+p, x]
    s8 = nc.declare_dram_parameter("s8", [npair * P, 2 * mhalf], f8,
                                   isOutput=False)
    t8 = nc.declare_dram_parameter("t8", [npair * P, 2 * n], f8,
                                   isOutput=False)
    vecs = nc.declare_dram_parameter("vecs", [2, n], f32, isOutput=True)
    r1o = nc.declare_dram_parameter("r1o", [P, nt_m], f32, isOutput=True)

    with tile.TileContext(nc) as tc:
        with (
            tc.tile_pool(name="persist", bufs=1) as pp,
            tc.tile_pool(name="opt", bufs=4) as optp,      # t8 pair blocks
            tc.tile_pool(name="ops", bufs=4) as opsp,      # s8 pair blocks
            tc.tile_pool(name="sw", bufs=3) as swpool,     # simw row tiles
            tc.tile_pool(name="kp", bufs=3) as kpool,      # K row tiles
            tc.tile_pool(name="xp", bufs=3) as xpool,      # X row tiles
            tc.tile_pool(name="prol", bufs=2) as prolp,    # squares scratch
            tc.tile_pool(name="vec", bufs=1) as vecp,      # [1,N] vectors
            tc.tile_pool(name="sm", bufs=16) as smp,       # tiny per-tile stats
            tc.tile_pool(name="acc", bufs=4, space="PSUM") as accp,
            tc.tile_pool(name="ps", bufs=4, space="PSUM") as psp,
        ):
            # ---- persistent sbuf ----
            t8b = [optp.tile([P, 2 * n], f8, name=f"t8b{q}", tag="opt")
                   for q in range(npair)]
            s8b = [opsp.tile([P, 2 * mhalf], f8, name=f"s8b{q}", tag="ops")
                   for q in range(npair)]
            invt_bc = pp.tile([P, n], bf16, name="invt_bc", tag="invt_bc")
            ones = pp.tile([P, 1], bf16, name="ones", tag="ones")
            inv_s = pp.tile([P, nt_m], f32, name="inv_s", tag="inv_s")
            scale10 = pp.tile([P, nt_m], f32, name="scale10", tag="scale10")
            r1buf = pp.tile([P, nt_m], f32, name="r1buf", tag="r1buf")
            biasm10 = pp.tile([P, 1], f32, name="biasm10", tag="biasm10")

            bias_ln10 = pp.tile([P, 1], f32, name="bias_ln10",
                                tag="bias_ln10")
            nc.vector.memset(biasm10[:], -10.0)
            nc.vector.memset(bias_ln10[:], float(np.log(10.0)))
            nc.vector.memset(ones[:], 1.0)

            # persistent Y/Z accumulator banks: matmul outputs may only
            # target base partitions {0,32,64,96}; chunks are spread so
            # that 4 consecutive accum matmuls hit 4 different 32-col PE
            # strips (they then co-execute). Zeroed by DVE, all matmuls
            # start=False -> first-touch accumulate is safe.
            pYZ = [accp.tile([P, CH], f32, name=f"pYZ{k}", tag="acc")
                   for k in range(4)]
            for k in range(4):
                nc.vector.memset(pYZ[k][:], 0.0)

            def yz_slot(c, is_z):
                t_ = pYZ[(2 if is_z else 0) + c % 2]
                off = 32 * (c // 2)
                return t_[off:off + 1, :], off

            # ---- input loads: aggregate HBM rate is fixed (~240 GB/s),
            # so ordering across the three DMA-capable queues just
            # controls which tiles land first; latest-needed go last.
            nc.scalar.dma_start(s8b[0][:], s8[ts(0, P), :])
            nc.sync.dma_start(t8b[0][:], t8[ts(0, P), :])
            nc.gpsimd.dma_start(t8b[1][:], t8[ts(1, P), :])
            nc.scalar.dma_start(s8b[1][:], s8[ts(1, P), :])
            nc.sync.dma_start(t8b[2][:], t8[ts(2, P), :])
            nc.gpsimd.dma_start(s8b[3][:], s8[ts(3, P), :])
            nc.scalar.dma_start(s8b[2][:], s8[ts(2, P), :])
            nc.sync.dma_start(t8b[3][:], t8[ts(3, P), :])

            # ---- norms: colsum(x*x) via squares + ones-matmuls. t-norms
            # pack 4 chunks per psum bank at partitions {0,32,64,96};
            # s-norms use the square chunk as the STATIONARY with a ones
            # column moving, landing colsum+transpose in one step
            # directly in the [128, nt_m] scale-AP layout. The 16
            # half-squares are assigned to engines in tile-arrival order
            # so no engine queue ever waits on a late DMA.
            pnorm = psp.tile([P, nt_m], f32, name="pnorm", tag="ps")
            nc.vector.memset(pnorm[:], 0.0)
            pnt = [psp.tile([P, CH], f32, name=f"pnt{k}", tag="ps")
                   for k in range(2)]
            for k in range(2):
                nc.vector.memset(pnt[k][:], 0.0)

            def square_half(kind, q, two, eng, last):
                if kind == "s":
                    sq = prolp.tile([P, mhalf], bf16, name="sqs",
                                    tag="prols")
                    half = s8b[q][:, two * mhalf:(two + 1) * mhalf]
                else:
                    sq = prolp.tile([P, n], bf16, name="sqt", tag="prol")
                    half = t8b[q][:, two * n:(two + 1) * n]
                if eng is nc.scalar:
                    nc.scalar.activation(sq[:], half, Act.Square)
                else:
                    eng.tensor_mul(sq[:], half, half)
                if kind == "s":
                    for j in range(nt_m):
                        nc.tensor.matmul(
                            pnorm[:, j:j + 1], sq[:, ts(j, P)], ones[:],
                            start=False, stop=last,
                            skip_group_check=True)
                else:
                    for c in range(nch):
                        slot = pnt[c // 4][32 * (c % 4):32 * (c % 4) + 1, :]
                        nc.tensor.matmul(
                            slot, ones[:], sq[:, ts(c, CH)],
                            start=False, stop=last,
                            skip_group_check=True,
                            tile_position=(0, 32 * (c % 4)))

            ACT, DVE, GPS = nc.scalar, nc.vector, nc.gpsimd
            plan = [
                ("s", 0, 0, GPS), ("s", 0, 1, GPS),
                ("t", 0, 0, DVE), ("t", 0, 1, DVE),
                ("t", 1, 0, DVE), ("t", 1, 1, DVE),
                ("s", 1, 0, ACT), ("s", 1, 1, ACT),
                ("t", 2, 0, DVE), ("t", 2, 1, ACT),
                ("s", 2, 0, ACT), ("s", 2, 1, ACT),
                ("s", 3, 0, GPS), ("s", 3, 1, GPS),
                ("t", 3, 0, DVE), ("t", 3, 1, ACT),
            ]
            s_left = sum(1 for k, *_ in plan if k == "s")
            t_left = sum(1 for k, *_ in plan if k == "t")
            for kind, q, two, eng in plan:
                if kind == "s":
                    s_left -= 1
                    square_half(kind, q, two, eng, s_left == 0)
                else:
                    t_left -= 1
                    square_half(kind, q, two, eng, t_left == 0)



            # inv = exp(-0.5*ln(x)+b) with ALL Ln ops batched before ALL
            # Exp ops -- Ln and Exp live in different ACT table sets and
            # each alternation costs a 1.3us ACT_TABLE_LOAD. b=ln(10)
            # bakes in the x10 exp scale.
            lns = smp.tile([P, nt_m], f32, name="lns", tag="smln")
            lnt = vecp.tile([1, n], f32, name="lnt", tag="veclnt")
            for c in range(nch):
                slot = pnt[c // 4][32 * (c % 4):32 * (c % 4) + 1, :]
                nc.scalar.activation(lnt[0:1, ts(c, CH)], slot, Act.Ln)
            nc.scalar.activation(lns[:], pnorm[:], Act.Ln)
            nc.scalar.activation(inv_s[:], lns[:], Act.Exp, scale=-0.5)
            nc.scalar.activation(scale10[:], lns[:], Act.Exp, scale=-0.5,
                                 bias=bias_ln10[:])
            invt_h = vecp.tile([1, n], bf16, name="invt_h", tag="vech")
            for c in range(nch):
                nc.scalar.activation(invt_h[0:1, ts(c, CH)],
                                     lnt[0:1, ts(c, CH)], Act.Exp,
                                     scale=-0.5)
            # broadcast invt down partitions via PE ones-matmuls
            onesrow = pp.tile([1, P], bf16, name="onesrow", tag="onesrow")
            nc.vector.memset(onesrow[:], 1.0)
            for c in range(nch):
                pb = psp.tile([P, CH], f32, name="pb", tag="ps")
                nc.tensor.matmul(pb[:], onesrow[:], invt_h[0:1, ts(c, CH)],
                                 start=True, stop=True)
                nc.vector.tensor_copy(invt_bc[:, ts(c, CH)], pb[:])

            # 3D pair views for DoubleRow: [128, two, x]
            s8v = [s8b[q].rearrange("p (two m) -> p two m", two=2)
                   for q in range(npair)]
            t8v = [t8b[q].rearrange("p (two x) -> p two x", two=2)
                   for q in range(npair)]

            # ---- fused single pass over row tiles ----
            # Y/Z accum matmuls for tile mi are issued after tile mi+1's
            # sim matmuls (1-stage software pipeline) so the in-order PE
            # queue never waits on the ACT->DVE r1 chain.
            prev = None

            # chunk order 0,2,4,6,1,3,5,7: consecutive matmuls land on
            # strips 0/32/64/96 of one bank and co-execute on the PE
            accum_order = [0, 2, 4, 6, 1, 3, 5, 7]

            def issue_accums(st):
                K_p, X_p, r1h_p, r1a_p, last = st
                for c in accum_order:
                    slot, off = yz_slot(c, False)
                    nc.tensor.matmul(slot, r1h_p[:], K_p[:, ts(c, CH)],
                                     start=False, stop=last,
                                     skip_group_check=True,
                                     tile_position=(0, off))
                for c in accum_order:
                    slot, off = yz_slot(c, True)
                    nc.tensor.matmul(slot, r1a_p[:], X_p[:, ts(c, CH)],
                                     start=False, stop=last,
                                     skip_group_check=True,
                                     tile_position=(0, off))

            # post-exp DVE work (X, r1 chain) for tile mi is issued after
            # tile mi+1's scale-mults, so the in-order DVE queue never
            # waits on the ACT exp; accums trail one further stage.
            def do_post(st):
                simw_p, K_p, rowsum_p, mi_p = st
                X_t = xpool.tile([P, n], bf16, name="X", tag="xp")
                # GpSimd X keeps DVE free mid-loop, but its 7.5us op
                # would sit on the drain critical path for the last tiles
                if mi_p >= nt_m - 2:
                    nc.vector.tensor_mul(X_t[:], K_p[:], simw_p[:])
                else:
                    nc.gpsimd.tensor_mul(X_t[:], K_p[:], simw_p[:])
                # r1' = 1/rowsum (u factor folded into the host epilogue)
                nc.vector.reciprocal(r1buf[:, mi_p:mi_p + 1], rowsum_p[:])
                r1h = smp.tile([P, 1], bf16, name="r1h", tag="smh")
                nc.vector.tensor_copy(r1h[:], r1buf[:, mi_p:mi_p + 1])
                r1a = smp.tile([P, 1], bf16, name="r1a", tag="smh")
                nc.vector.tensor_mul(r1a[:], r1buf[:, mi_p:mi_p + 1],
                                     inv_s[:, mi_p:mi_p + 1])
                return (K_p, X_t, r1h, r1a, mi_p == nt_m - 1)

            pend = None
            for mi in range(nt_m):
                simw = swpool.tile([P, n], bf16, name="simw", tag="sw")
                K_t = kpool.tile([P, n], bf16, name="K", tag="kp")
                for ni in range(nch):
                    pm = psp.tile([P, CH], f32, name="pm", tag="ps")
                    for q in range(npair):
                        nc.tensor.matmul(
                            pm[:],
                            s8v[q][:, :, ts(mi, P)],
                            t8v[q][:, :, ts(ni, CH)],
                            start=(q == 0), stop=(q == npair - 1),
                            perf_mode=DR)
                    nc.vector.tensor_mul(simw[:, ts(ni, CH)], pm[:],
                                         invt_bc[:, ts(ni, CH)])
                rowsum = smp.tile([P, 1], f32, name="rowsum", tag="sm")
                nc.scalar.activation(K_t[:], simw[:], Act.Exp,
                                     bias=biasm10[:],
                                     scale=scale10[:, mi:mi + 1],
                                     accum_out=rowsum[:])
                if pend is not None:
                    st = do_post(pend)
                    if prev is not None:
                        issue_accums(prev)
                    prev = st
                pend = (simw, K_t, rowsum, mi)
            st = do_post(pend)
            if prev is not None:
                issue_accums(prev)
            issue_accums(st)

            # ---- outputs (engine writes must start at partition 0/32);
            # Y copies on ACT, Z copies on DVE, into separate tiles so
            # the two chains don't serialize on write tracking ----
            nc.gpsimd.dma_start(r1o[:, :], r1buf[:])
            yt = vecp.tile([1, n], f32, name="yt", tag="veclnt")
            zt = vecp.tile([1, n], f32, name="zt", tag="vec")
            for c in range(nch):
                nc.scalar.copy(yt[0:1, ts(c, CH)], yz_slot(c, False)[0])
                nc.vector.tensor_copy(zt[0:1, ts(c, CH)],
                                      yz_slot(c, True)[0])
            nc.sync.dma_start(vecs[0:1, :], yt[0:1, :])
            nc.sync.dma_start(vecs[1:2, :], zt[0:1, :])

    return nc


def _pair_interleave(xT):
    """[D, F] -> [D//256*128, 2*F] fp8 with row q*128+p, col two*F+x."""
    dd, f = xT.shape
    return np.ascontiguousarray(
        xT.reshape(dd // 256, 2, P, f).transpose(0, 2, 1, 3)
        .reshape(dd // 2, 2 * f)).astype(F8)


def _make_in_maps(gts, feats):
    in_maps = []
    for core in range(NCORES):
        e, h = divmod(core, 2)
        s_half = gts[e][h * MHALF:(h + 1) * MHALF]          # [2048, 1024]
        in_maps.append({
            "s8": _pair_interleave(np.ascontiguousarray(s_half.T)),
            "t8": _pair_interleave(np.ascontiguousarray(feats[e].T)),
        })
    return in_maps


def _ensemble(losses, prev_losses):
    l = np.asarray(losses, np.float64)
    ratio = l / (np.asarray(prev_losses, np.float64) + 1e-8)
    w = np.exp(ratio / 1.0)
    w = w / np.sum(w) * l.shape[0]
    return np.float32(np.sum(w * l))


def _numpy_reference(gts, feats, prev_losses):
    """Faithful float32 fallback, used only if the on-device convergence
    check is violated (never observed for this problem's regime)."""
    losses = []
    for e in range(gts.shape[0]):
        s = gts[e] / np.maximum(
            np.linalg.norm(gts[e], axis=1, keepdims=True), 1e-12)
        t = feats[e] / np.maximum(
            np.linalg.norm(feats[e], axis=1, keepdims=True), 1e-12)
        sim = (s @ t.T).astype(np.float32)
        K = np.exp(-(1.0 - sim) / 0.1)
        m, n = sim.shape
        u = np.full(m, 1.0 / m, np.float32)
        v = np.full(n, 1.0 / n, np.float32)
        r = np.ones(m, np.float32)
        c = np.ones(n, np.float32)
        err = np.inf
        for _ in range(100):
            if err < 0.01:
                break
            r_new = u / (K @ c)
            c = v / (K.T @ r_new)
            err = float(np.mean(np.abs(r_new - r)))
            r = r_new
        losses.append(np.sum(np.outer(r, c) * K * sim))
    return _ensemble(losses, prev_losses)


def _run(gts, feats, trace=False):
    from concourse.bass_utils import run_bass_kernel_spmd
    if "nc" not in _CACHE:
        nc = build_bass()
        nc.finalize()
        _CACHE["nc"] = nc
    in_maps = _make_in_maps(gts, feats)
    return run_bass_kernel_spmd(_CACHE["nc"], in_maps,
                                list(range(NCORES)), trace=trace)


def _sampled_sinkhorn_check(gts, feats, r1_full, c1_full, n_samples=4):
    """Estimate mean|r2 - r1| from a few exact host-side rows; also require
    err1 = mean|r1 - 1| >= 0.01 (else the reference stops at iteration 1,
    where our formula is exact anyway)."""
    rng = np.random.default_rng(0)
    u = 1.0 / M
    for e in range(E):
        r1 = r1_full[e]
        err1 = np.mean(np.abs(r1 - 1.0))
        if err1 < 0.01:
            continue  # 1-iteration stop: our plan is the exact one
        t = feats[e].astype(np.float64)
        t /= np.maximum(np.linalg.norm(t, axis=1, keepdims=True), 1e-12)
        rows = rng.choice(M, size=n_samples, replace=False)
        s_rows = gts[e][rows].astype(np.float64)
        s_rows /= np.maximum(
            np.linalg.norm(s_rows, axis=1, keepdims=True), 1e-12)
        sim_rows = s_rows @ t.T
        K_rows = np.exp((sim_rows - 1.0) * 10.0)
        r2_rows = u / (K_rows @ c1_full[e])
        if np.mean(np.abs(r2_rows - r1[rows])) >= 0.005:
            return False
    return True


def _combine(results, gts, feats, prev_losses):
    losses = []
    r1_full = []
    c1_full = []
    u = 1.0 / M
    v = 1.0 / N
    for e in range(E):
        a, b = results[2 * e], results[2 * e + 1]
        Y1 = a["vecs"][0].astype(np.float64) + b["vecs"][0].astype(np.float64)
        Z = a["vecs"][1].astype(np.float64) + b["vecs"][1].astype(np.float64)
        # device stores r1' = 1/rowsum and u-free partials; u cancels in
        # c1*Z but is needed for the convergence guard quantities.
        losses.append(v * np.sum(Z / Y1))
        r1 = u * np.concatenate([a["r1o"].T.reshape(-1),
                                 b["r1o"].T.reshape(-1)])
        r1_full.append(r1)
        c1_full.append(v / (u * Y1))
    if not _sampled_sinkhorn_check(gts, feats, r1_full, c1_full):
        return _numpy_reference(gts, feats, prev_losses)
    return _ensemble(losses, prev_losses)


def kernel(gts, feats, prev_losses):
    gts = np.asarray(gts, np.float32)
    feats = np.asarray(feats, np.float32)
    prev_losses = np.asarray(prev_losses, np.float32)
    res = _run(gts, feats)
    return _combine(res.results, gts, feats, prev_losses)


# revision 63
# speedup vs baseline: 1.0104x; 1.0104x over previous
"""Trainium2 Bass kernel for the EnsembleFeatureLoss OT problem.

Math (per ensemble member e of E=4):
  s = l2norm_rows(gts[e]); t = l2norm_rows(feats[e])      # [4096, 1024]
  sim = s @ t.T                                            # [4096, 4096]
  K = exp(10*sim - 10)
  Sinkhorn on this data converges at iteration 1 (fp64: mean|r2-r1| ~ 5e-8,
  loss delta 6e-8 rel), so the transport plan is outer(r1, c1) * K with
    r1 = u / rowsum(K),  c1 = v / Y1,  Y1 = K^T r1
  and
    loss = sum_n c1[n] * Z[n],  Z[n] = sum_m r1[m] K[m,n] sim[m,n]
  A host-side sampled check of |r2 - r1| guards the approximation; on
  violation we fall back to a faithful numpy reference.

Distribution: 8 cores = 4 members x 2 row-halves (2048 rows each).
One fused pass per core over its [2048, 4096] block:
  fp8e4 DoubleRow matmul (2 d-blocks per instruction, 2 MACs/cell/cycle)
  -> psum chunk -> DVE applies 1/|t| (bf16 simw) -> one big ACT exp per
  row tile (per-row scale AP carries 10/|s|, fused rowsum accum) -> r1 ->
  PE accumulates Y1 (r1-weighted colsums of K) and Z (r1/|s|-weighted
  colsums of K*simw) into partition-packed psum banks across all 16 row
  tiles. No DRAM spill, no second pass, no device collective: each core
  outputs Y1/Z partials [2, 4096] + 1/rowsum; the host adds the two
  half partials and does the O(N) epilogue.

Inputs are shipped as raw (unnormalized) fp8e4 pair-interleaved blocks
(6 MB/core); both norm vectors are computed on device from the fp8
operands (squares -> PE ones-matmul colsums; the s-norm [1,2048] row is
moved into the per-row [128,16] scale layout with 16 PE transposes).
"""

import numpy as np
import ml_dtypes

BF16 = ml_dtypes.bfloat16
F8 = ml_dtypes.float8_e4m3

E = 4
M = 4096
N = 4096
D = 1024
P = 128
NCORES = 8
MHALF = M // 2              # rows per core
CH = 512                    # psum chunk (one fp32 bank)

_CACHE = {}


def build_bass(mhalf=MHALF, n=N, d=D, ncores=NCORES):
    import concourse.bass as bass
    import concourse.mybir as mybir
    import concourse.tile as tile
    from concourse import bacc
    from concourse.bass import ts

    dt = mybir.dt
    f32, bf16, f8 = dt.float32, dt.bfloat16, dt.float8e4
    Act = mybir.ActivationFunctionType
    DR = mybir.MatmulPerfMode.DoubleRow

    nt_m = mhalf // P           # 16 row tiles
    nd = d // P                 # 8 contraction blocks
    npair = nd // 2             # 4 DoubleRow pair blocks
    nch = n // CH               # 8 column chunks
    mch = mhalf // CH           # 4 s-norm column chunks

    nc = bacc.Bacc("TRN2", target_bir_lowering=False, debug=False,
                   num_devices=ncores)
    # pair-interleaved fp8 operands: row q*128+p, col two*F+x holds
    # X[(2q+two)*128+p, x]
    s8 = nc.declare_dram_parameter("s8", [npair * P, 2 * mhalf], f8,
                                   isOutput=False)
    t8 = nc.declare_dram_parameter("t8", [npair * P, 2 * n], f8,
                                   isOutput=False)
    vecs = nc.declare_dram_parameter("vecs", [2, n], f32, isOutput=True)
    r1o = nc.declare_dram_parameter("r1o", [P, nt_m], f32, isOutput=True)

    with tile.TileContext(nc) as tc:
        with (
            tc.tile_pool(name="persist", bufs=1) as pp,
            tc.tile_pool(name="opt", bufs=4) as optp,      # t8 pair blocks
            tc.tile_pool(name="ops", bufs=4) as opsp,      # s8 pair blocks
            tc.tile_pool(name="sw", bufs=3) as swpool,     # simw row tiles
            tc.tile_pool(name="kp", bufs=3) as kpool,      # K row tiles
            tc.tile_pool(name="xp", bufs=3) as xpool,      # X row tiles
            tc.tile_pool(name="prol", bufs=2) as prolp,    # squares scratch
            tc.tile_pool(name="vec", bufs=1) as vecp,      # [1,N] vectors
            tc.tile_pool(name="sm", bufs=16) as smp,       # tiny per-tile stats
            tc.tile_pool(name="acc", bufs=4, space="PSUM") as accp,
            tc.tile_pool(name="ps", bufs=4, space="PSUM") as psp,
        ):
            # ---- persistent sbuf ----
            t8b = [optp.tile([P, 2 * n], f8, name=f"t8b{q}", tag="opt")
                   for q in range(npair)]
            s8b = [opsp.tile([P, 2 * mhalf], f8, name=f"s8b{q}", tag="ops")
                   for q in range(npair)]
            invt_bc = pp.tile([P, n], bf16, name="invt_bc", tag="invt_bc")
            ones = pp.tile([P, 1], bf16, name="ones", tag="ones")
            ident1 = pp.tile([1, 1], f32, name="ident1", tag="ident1")
            inv_s = pp.tile([P, nt_m], f32, name="inv_s", tag="inv_s")
            scale10 = pp.tile([P, nt_m], f32, name="scale10", tag="scale10")
            r1buf = pp.tile([P, nt_m], f32, name="r1buf", tag="r1buf")
            biasm10 = pp.tile([P, 1], f32, name="biasm10", tag="biasm10")

            bias_ln10 = pp.tile([P, 1], f32, name="bias_ln10",
                                tag="bias_ln10")
            nc.vector.memset(biasm10[:], -10.0)
            nc.vector.memset(bias_ln10[:], float(np.log(10.0)))
            nc.vector.memset(ones[:], 1.0)
            nc.vector.memset(ident1[:], 1.0)

            # persistent Y/Z accumulator banks: matmul outputs may only
            # target base partitions {0,32,64,96}; chunks are spread so
            # that 4 consecutive accum matmuls hit 4 different 32-col PE
            # strips (they then co-execute). Zeroed by DVE, all matmuls
            # start=False -> first-touch accumulate is safe.
            pYZ = [accp.tile([P, CH], f32, name=f"pYZ{k}", tag="acc")
                   for k in range(4)]
            for k in range(4):
                nc.vector.memset(pYZ[k][:], 0.0)

            def yz_slot(c, is_z):
                t_ = pYZ[(2 if is_z else 0) + c % 2]
                off = 32 * (c // 2)
                return t_[off:off + 1, :], off

            # ---- input loads: aggregate HBM rate is fixed (~240 GB/s),
            # so ordering across the three DMA-capable queues just
            # controls which tiles land first; latest-needed go last.
            nc.scalar.dma_start(s8b[0][:], s8[ts(0, P), :])
            nc.sync.dma_start(t8b[0][:], t8[ts(0, P), :])
            nc.gpsimd.dma_start(t8b[1][:], t8[ts(1, P), :])
            nc.scalar.dma_start(s8b[1][:], s8[ts(1, P), :])
            nc.sync.dma_start(t8b[2][:], t8[ts(2, P), :])
            nc.gpsimd.dma_start(s8b[3][:], s8[ts(3, P), :])
            nc.scalar.dma_start(s8b[2][:], s8[ts(2, P), :])
            nc.sync.dma_start(t8b[3][:], t8[ts(3, P), :])

            # ---- norms: colsum(x*x) via squares + ones-matmuls, packed
            # 4 chunks per psum bank at partitions {0,32,64,96}. The 16
            # half-squares are assigned to engines in tile-arrival order
            # so no engine queue ever waits on a late DMA.
            psn = psp.tile([P, CH], f32, name="psn", tag="ps")
            nc.vector.memset(psn[:], 0.0)
            pnt = [psp.tile([P, CH], f32, name=f"pnt{k}", tag="ps")
                   for k in range(2)]
            for k in range(2):
                nc.vector.memset(pnt[k][:], 0.0)

            def square_half(kind, q, two, eng, last):
                if kind == "s":
                    sq = prolp.tile([P, mhalf], bf16, name="sqs",
                                    tag="prols")
                    half = s8b[q][:, two * mhalf:(two + 1) * mhalf]
                else:
                    sq = prolp.tile([P, n], bf16, name="sqt", tag="prol")
                    half = t8b[q][:, two * n:(two + 1) * n]
                if eng is nc.scalar:
                    nc.scalar.activation(sq[:], half, Act.Square)
                else:
                    eng.tensor_mul(sq[:], half, half)
                if kind == "s":
                    for c in range(mch):
                        nc.tensor.matmul(
                            psn[32 * c:32 * c + 1, :], ones[:],
                            sq[:, ts(c, CH)], start=False, stop=last,
                            skip_group_check=True,
                            tile_position=(0, 32 * c))
                else:
                    for c in range(nch):
                        slot = pnt[c // 4][32 * (c % 4):32 * (c % 4) + 1, :]
                        nc.tensor.matmul(
                            slot, ones[:], sq[:, ts(c, CH)],
                            start=False, stop=last,
                            skip_group_check=True,
                            tile_position=(0, 32 * (c % 4)))

            ACT, DVE, GPS = nc.scalar, nc.vector, nc.gpsimd
            plan = [
                ("s", 0, 0, GPS), ("s", 0, 1, GPS),
                ("t", 0, 0, DVE), ("t", 0, 1, DVE),
                ("t", 1, 0, DVE), ("t", 1, 1, DVE),
                ("s", 1, 0, ACT), ("s", 1, 1, ACT),
                ("t", 2, 0, DVE), ("t", 2, 1, ACT),
                ("s", 2, 0, ACT), ("s", 2, 1, ACT),
                ("s", 3, 0, GPS), ("s", 3, 1, GPS),
                ("t", 3, 0, DVE), ("t", 3, 1, ACT),
            ]
            s_left = sum(1 for k, *_ in plan if k == "s")
            t_left = sum(1 for k, *_ in plan if k == "t")
            for kind, q, two, eng in plan:
                if kind == "s":
                    s_left -= 1
                    square_half(kind, q, two, eng, s_left == 0)
                else:
                    t_left -= 1
                    square_half(kind, q, two, eng, t_left == 0)

            snorm = vecp.tile([1, mhalf], f32, name="snorm", tag="vecs")
            for c in range(mch):
                nc.vector.tensor_copy(snorm[0:1, ts(c, CH)],
                                      psn[32 * c:32 * c + 1, :])

            # s-norm layout transposes; then inv = exp(-0.5*ln(x)+b) with
            # ALL Ln ops batched before ALL Exp ops -- Ln and Exp live in
            # different ACT table sets and each alternation costs a
            # 1.3us ACT_TABLE_LOAD. b=ln(10) bakes in the x10 exp scale.
            pnorm = psp.tile([P, nt_m], f32, name="pnorm", tag="ps")
            for j in range(nt_m):
                nc.tensor.transpose(pnorm[:, j:j + 1],
                                    snorm[0:1, ts(j, P)], ident1[:])
            lns = smp.tile([P, nt_m], f32, name="lns", tag="smln")
            lnt = vecp.tile([1, n], f32, name="lnt", tag="veclnt")
            for c in range(nch):
                slot = pnt[c // 4][32 * (c % 4):32 * (c % 4) + 1, :]
                nc.scalar.activation(lnt[0:1, ts(c, CH)], slot, Act.Ln)
            nc.scalar.activation(lns[:], pnorm[:], Act.Ln)
            nc.scalar.activation(inv_s[:], lns[:], Act.Exp, scale=-0.5)
            nc.scalar.activation(scale10[:], lns[:], Act.Exp, scale=-0.5,
                                 bias=bias_ln10[:])
            invt_h = vecp.tile([1, n], bf16, name="invt_h", tag="vech")
            for c in range(nch):
                nc.scalar.activation(invt_h[0:1, ts(c, CH)],
                                     lnt[0:1, ts(c, CH)], Act.Exp,
                                     scale=-0.5)
            # broadcast invt down partitions via PE ones-matmuls
            onesrow = pp.tile([1, P], bf16, name="onesrow", tag="onesrow")
            nc.vector.memset(onesrow[:], 1.0)
            for c in range(nch):
                pb = psp.tile([P, CH], f32, name="pb", tag="ps")
                nc.tensor.matmul(pb[:], onesrow[:], invt_h[0:1, ts(c, CH)],
                                 start=True, stop=True)
                nc.vector.tensor_copy(invt_bc[:, ts(c, CH)], pb[:])

            # 3D pair views for DoubleRow: [128, two, x]
            s8v = [s8b[q].rearrange("p (two m) -> p two m", two=2)
                   for q in range(npair)]
            t8v = [t8b[q].rearrange("p (two x) -> p two x", two=2)
                   for q in range(npair)]

            # ---- fused single pass over row tiles ----
            # Y/Z accum matmuls for tile mi are issued after tile mi+1's
            # sim matmuls (1-stage software pipeline) so the in-order PE
            # queue never waits on the ACT->DVE r1 chain.
            prev = None

            # chunk order 0,2,4,6,1,3,5,7: consecutive matmuls land on
            # strips 0/32/64/96 of one bank and co-execute on the PE
            accum_order = [0, 2, 4, 6, 1, 3, 5, 7]

            def issue_accums(st):
                K_p, X_p, r1h_p, r1a_p, last = st
                for c in accum_order:
                    slot, off = yz_slot(c, False)
                    nc.tensor.matmul(slot, r1h_p[:], K_p[:, ts(c, CH)],
                                     start=False, stop=last,
                                     skip_group_check=True,
                                     tile_position=(0, off))
                for c in accum_order:
                    slot, off = yz_slot(c, True)
                    nc.tensor.matmul(slot, r1a_p[:], X_p[:, ts(c, CH)],
                                     start=False, stop=last,
                                     skip_group_check=True,
                                     tile_position=(0, off))

            # post-exp DVE work (X, r1 chain) for tile mi is issued after
            # tile mi+1's scale-mults, so the in-order DVE queue never
            # waits on the ACT exp; accums trail one further stage.
            def do_post(st):
                simw_p, K_p, rowsum_p, mi_p = st
                X_t = xpool.tile([P, n], bf16, name="X", tag="xp")
                # GpSimd X keeps DVE free mid-loop, but its 7.5us op
                # would sit on the drain critical path for the last tiles
                if mi_p >= nt_m - 2:
                    nc.vector.tensor_mul(X_t[:], K_p[:], simw_p[:])
                else:
                    nc.gpsimd.tensor_mul(X_t[:], K_p[:], simw_p[:])
                # r1' = 1/rowsum (u factor folded into the host epilogue)
                nc.vector.reciprocal(r1buf[:, mi_p:mi_p + 1], rowsum_p[:])
                r1h = smp.tile([P, 1], bf16, name="r1h", tag="smh")
                nc.vector.tensor_copy(r1h[:], r1buf[:, mi_p:mi_p + 1])
                r1a = smp.tile([P, 1], bf16, name="r1a", tag="smh")
                nc.vector.tensor_mul(r1a[:], r1buf[:, mi_p:mi_p + 1],
                                     inv_s[:, mi_p:mi_p + 1])
                return (K_p, X_t, r1h, r1a, mi_p == nt_m - 1)

            pend = None
            for mi in range(nt_m):
                simw = swpool.tile([P, n], bf16, name="simw", tag="sw")
                K_t = kpool.tile([P, n], bf16, name="K", tag="kp")
                for ni in range(nch):
                    pm = psp.tile([P, CH], f32, name="pm", tag="ps")
                    for q in range(npair):
                        nc.tensor.matmul(
                            pm[:],
                            s8v[q][:, :, ts(mi, P)],
                            t8v[q][:, :, ts(ni, CH)],
                            start=(q == 0), stop=(q == npair - 1),
                            perf_mode=DR)
                    nc.vector.tensor_mul(simw[:, ts(ni, CH)], pm[:],
                                         invt_bc[:, ts(ni, CH)])
                rowsum = smp.tile([P, 1], f32, name="rowsum", tag="sm")
                nc.scalar.activation(K_t[:], simw[:], Act.Exp,
                                     bias=biasm10[:],
                                     scale=scale10[:, mi:mi + 1],
                                     accum_out=rowsum[:])
                if pend is not None:
                    st = do_post(pend)
                    if prev is not None:
                        issue_accums(prev)
                    prev = st
                pend = (simw, K_t, rowsum, mi)
            st = do_post(pend)
            if prev is not None:
                issue_accums(prev)
            issue_accums(st)

            # ---- outputs (engine writes must start at partition 0/32);
            # Y copies on ACT, Z copies on DVE, into separate tiles so
            # the two chains don't serialize on write tracking ----
            nc.gpsimd.dma_start(r1o[:, :], r1buf[:])
            yt = vecp.tile([1, n], f32, name="yt", tag="veclnt")
            zt = vecp.tile([1, n], f32, name="zt", tag="vec")
            for c in range(nch):
                nc.scalar.copy(yt[0:1, ts(c, CH)], yz_slot(c, False)[0])
                nc.vector.tensor_copy(zt[0:1, ts(c, CH)],
                                      yz_slot(c, True)[0])
            nc.sync.dma_start(vecs[0:1, :], yt[0:1, :])
            nc.sync.dma_start(vecs[1:2, :], zt[0:1, :])

    return nc


def _pair_interleave(xT):
    """[D, F] -> [D//256*128, 2*F] fp8 with row q*128+p, col two*F+x."""
    dd, f = xT.shape
    return np.ascontiguousarray(
        xT.reshape(dd // 256, 2, P, f).transpose(0, 2, 1, 3)
        .reshape(dd // 2, 2 * f)).astype(F8)


def _make_in_maps(gts, feats):
    in_maps = []
    for core in range(NCORES):
        e, h = divmod(core, 2)
        s_half = gts[e][h * MHALF:(h + 1) * MHALF]          # [2048, 1024]
        in_maps.append({
            "s8": _pair_interleave(np.ascontiguousarray(s_half.T)),
            "t8": _pair_interleave(np.ascontiguousarray(feats[e].T)),
        })
    return in_maps


def _ensemble(losses, prev_losses):
    l = np.asarray(losses, np.float64)
    ratio = l / (np.asarray(prev_losses, np.float64) + 1e-8)
    w = np.exp(ratio / 1.0)
    w = w / np.sum(w) * l.shape[0]
    return np.float32(np.sum(w * l))


def _numpy_reference(gts, feats, prev_losses):
    """Faithful float32 fallback, used only if the on-device convergence
    check is violated (never observed for this problem's regime)."""
    losses = []
    for e in range(gts.shape[0]):
        s = gts[e] / np.maximum(
            np.linalg.norm(gts[e], axis=1, keepdims=True), 1e-12)
        t = feats[e] / np.maximum(
            np.linalg.norm(feats[e], axis=1, keepdims=True), 1e-12)
        sim = (s @ t.T).astype(np.float32)
        K = np.exp(-(1.0 - sim) / 0.1)
        m, n = sim.shape
        u = np.full(m, 1.0 / m, np.float32)
        v = np.full(n, 1.0 / n, np.float32)
        r = np.ones(m, np.float32)
        c = np.ones(n, np.float32)
        err = np.inf
        for _ in range(100):
            if err < 0.01:
                break
            r_new = u / (K @ c)
            c = v / (K.T @ r_new)
            err = float(np.mean(np.abs(r_new - r)))
            r = r_new
        losses.append(np.sum(np.outer(r, c) * K * sim))
    return _ensemble(losses, prev_losses)


def _run(gts, feats, trace=False):
    from concourse.bass_utils import run_bass_kernel_spmd
    if "nc" not in _CACHE:
        nc = build_bass()
        nc.finalize()
        _CACHE["nc"] = nc
    in_maps = _make_in_maps(gts, feats)
    return run_bass_kernel_spmd(_CACHE["nc"], in_maps,
                                list(range(NCORES)), trace=trace)


def _sampled_sinkhorn_check(gts, feats, r1_full, c1_full, n_samples=4):
    """Estimate mean|r2 - r1| from a few exact host-side rows; also require
    err1 = mean|r1 - 1| >= 0.01 (else the reference stops at iteration 1,
    where our formula is exact anyway)."""
    rng = np.random.default_rng(0)
    u = 1.0 / M
    for e in range(E):
        r1 = r1_full[e]
        err1 = np.mean(np.abs(r1 - 1.0))
        if err1 < 0.01:
            continue  # 1-iteration stop: our plan is the exact one
        t = feats[e].astype(np.float64)
        t /= np.maximum(np.linalg.norm(t, axis=1, keepdims=True), 1e-12)
        rows = rng.choice(M, size=n_samples, replace=False)
        s_rows = gts[e][rows].astype(np.float64)
        s_rows /= np.maximum(
            np.linalg.norm(s_rows, axis=1, keepdims=True), 1e-12)
        sim_rows = s_rows @ t.T
        K_rows = np.exp((sim_rows - 1.0) * 10.0)
        r2_rows = u / (K_rows @ c1_full[e])
        if np.mean(np.abs(r2_rows - r1[rows])) >= 0.005:
            return False
    return True


def _combine(results, gts, feats, prev_losses):
    losses = []
    r1_full = []
    c1_full = []
    u = 1.0 / M
    v = 1.0 / N
    for e in range(E):
        a, b = results[2 * e], results[2 * e + 1]
        Y1 = a["vecs"][0].astype(np.float64) + b["vecs"][0].astype(np.float64)
        Z = a["vecs"][1].astype(np.float64) + b["vecs"][1].astype(np.float64)
        # device stores r1' = 1/rowsum and u-free partials; u cancels in
        # c1*Z but is needed for the convergence guard quantities.
        losses.append(v * np.sum(Z / Y1))
        r1 = u * np.concatenate([a["r1o"].T.reshape(-1),
                                 b["r1o"].T.reshape(-1)])
        r1_full.append(r1)
        c1_full.append(v / (u * Y1))
    if not _sampled_sinkhorn_check(gts, feats, r1_full, c1_full):
        return _numpy_reference(gts, feats, prev_losses)
    return _ensemble(losses, prev_losses)


def kernel(gts, feats, prev_losses):
    gts = np.asarray(gts, np.float32)
    feats = np.asarray(feats, np.float32)
    prev_losses = np.asarray(prev_losses, np.float32)
    res = _run(gts, feats)
    return _combine(res.results, gts, feats, prev_losses)
